# revision 1
# baseline (speedup 1.0000x reference)
"""Trainium2 Bass kernel for multi-head attention (QKV proj + RoPE + softmax attention + out proj).

Problem: x[2,2048,2048], wq/wk/wv/wo[2048,2048], 16 heads x 128 dim, start_pos=0,
KV cache is fully overwritten before being read, so it never affects the output.

Sharding: 8 cores = 2 batches x 4 head-groups (4 heads each).  Each core computes
partial output  attn_heads(x[b]) @ woT[:, group]  and the host sums the 4 group
partials per batch.

Device-side layout strategy (all matmuls contract over the partition dim):
  - host supplies xT = x[b].T                       [E, T]
  - QT = (wqT)^T-matmuls -> [heads*128, T]  (head-dim on partitions)
  - scores^T tile = KT_tile^T-mm  -> softmax-exp without max subtraction
    (scores ~ N(0,1), max |score| < ~7, exp is safe in fp32)
  - PV uses P^T directly; row-sums via ones-matrix matmul (which also
    partition-broadcasts them for free); normalization is applied to the
    small attention output, not to P.
  - RoPE: host permutes wq/wk out-rows so each head-pair's (real, imag) halves
    land in two full 128-partition tiles -> rotation is 6 full-width DVE ops.
  - float32r matmuls (1 cycle/row at N=512 vs 4 for fp32), rel err ~2e-4.
  - DMA loads are sliced so consumers only wait for their own k-slices; QKV
    round-trips through DRAM scratch; head 0's K/V are prefetched into
    persistent SBUF during phase A so attention starts right at the boundary.
"""

import math
import sys

sys.path.insert(0, "/opt/trn_rl_repo")

import numpy as np

import concourse.bacc as bacc
import concourse.mybir as mybir
import concourse.tile as tile
from concourse.bass_utils import run_bass_kernel_spmd

P = 128
F32 = mybir.dt.float32
F32R = mybir.dt.float32r
MUL = mybir.AluOpType.mult
SUB = mybir.AluOpType.subtract
ADD = mybir.AluOpType.add


def build_attention_nc(T, E, HL, HD=128, CH=512):
    """One-core program: HL local heads, seq len T, embed E (full), head dim HD=128.

    Inputs (per core): xT[E,T], wqT/wkT/wvT[E,HL*HD] (pair-permuted for q/k),
    woT[HL*HD,E], cosF/sinF[P,T].  Output: outp[T,E] (partial, sum over groups).
    """
    assert HD == P and E % P == 0 and T % P == 0 and T % CH == 0
    assert HL % 2 == 0 and HL * HD <= 512 and CH <= 512
    ET = E // P          # contraction tiles for the projections
    TC = T // CH         # t-chunks
    ST = T // P          # s-tiles
    D2 = HD // 2
    NPAIR = HL // 2
    scale = 1.0 / math.sqrt(HD)

    nc = bacc.Bacc(None)
    xT = nc.dram_tensor("xT", [E, T], F32R, kind="ExternalInput")
    wqT = nc.dram_tensor("wqT", [E, HL * HD], F32R, kind="ExternalInput")
    wkT = nc.dram_tensor("wkT", [E, HL * HD], F32R, kind="ExternalInput")
    wvT = nc.dram_tensor("wvT", [E, HL * HD], F32R, kind="ExternalInput")
    woT = nc.dram_tensor("woT", [HL * HD, E], F32R, kind="ExternalInput")
    cosF = nc.dram_tensor("cosF", [P, T], F32, kind="ExternalInput")
    sinF = nc.dram_tensor("sinF", [P, T], F32, kind="ExternalInput")
    outp = nc.dram_tensor("outp", [T, E], F32, kind="ExternalOutput")

    xT_t = xT.rearrange("(o p) t -> p o t", p=P)
    wq_t = wqT.rearrange("(o p) m -> p o m", p=P)
    wk_t = wkT.rearrange("(o p) m -> p o m", p=P)
    wv_t = wvT.rearrange("(o p) m -> p o m", p=P)
    wo_t = woT.rearrange("(h p) e -> p h e", p=P)

    with tile.TileContext(nc) as tc:
        with (
            tc.tile_pool(name="dram", bufs=1, space="DRAM") as dram,
            tc.tile_pool(name="keep", bufs=1) as keep,
        ):
            qt_scr = dram.tile([HL * HD, T], F32R)
            kt_scr = dram.tile([HL * HD, T], F32R)
            v_scr = dram.tile([T, HL * HD], F32R)
            v_scr_t = v_scr.rearrange("(o p) d -> p o d", p=P)
            # head 0's K/V prefetched into persistent SBUF during phase A so
            # attention can start the moment phase A's PE work finishes.
            kt0_sb = keep.tile([P, T], F32R)
            v0_sb = keep.tile([P, ST, HD], F32R)
            qt0_sb = keep.tile([P, T], F32R)
            ones_sb = keep.tile([P, P], F32R)

            # ---------------- Phase A: QKV projections + RoPE ----------------
            with (
                tc.tile_pool(name="aw", bufs=1) as aw,
                tc.tile_pool(name="ax", bufs=2) as ax,
                tc.tile_pool(name="acs", bufs=1) as acs,
                tc.tile_pool(name="aps", bufs=2, space="PSUM") as aps,
                tc.tile_pool(name="arot", bufs=2) as arot,
            ):
                warm_f = aw.tile([P, 64], F32)
                nc.vector.memset(warm_f[:], 0.0)
                warm = aw.tile([P, 64], F32R)
                nc.vector.tensor_copy(warm[:], warm_f[:])
                ones_f32 = aw.tile([P, P], F32)
                nc.vector.memset(ones_f32[:], 1.0)
                nc.vector.tensor_copy(ones_sb[:], ones_f32[:])
                wq_sb = aw.tile([P, ET, HL * HD], F32R)
                wk_sb = aw.tile([P, ET, HL * HD], F32R)
                wv_sb = aw.tile([P, ET, HL * HD], F32R)
                x0_sb = ax.tile([P, ET, CH], F32R, tag="xc")
                # interleaved k-sliced loads: the k-th matmul of the first
                # accumulation only waits for its own slices.
                for k2 in range(0, ET, 2):
                    nc.sync.dma_start(wq_sb[:, k2:k2 + 2, :], wq_t[:, k2:k2 + 2, :])
                    nc.sync.dma_start(x0_sb[:, k2:k2 + 2, :], xT_t[:, k2:k2 + 2, 0:CH])
                for k2 in range(0, ET, 2):
                    nc.sync.dma_start(wk_sb[:, k2:k2 + 2, :], wk_t[:, k2:k2 + 2, :])
                for k2 in range(0, ET, 2):
                    nc.sync.dma_start(wv_sb[:, k2:k2 + 2, :], wv_t[:, k2:k2 + 2, :])

                with tc.tile_pool(name="wps", bufs=1, space="PSUM") as wps:
                    wp = wps.tile([64, 64], F32)
                    for _ in range(80):
                        nc.tensor.matmul(wp[:], warm[:, 0:64], warm[:, 0:64],
                                         start=True, stop=True)

                for c in range(TC):
                    csl = slice(c * CH, (c + 1) * CH)
                    ssl = slice(c * (CH // P), (c + 1) * (CH // P))
                    if c == 0:
                        x_sb = x0_sb
                    else:
                        x_sb = ax.tile([P, ET, CH], F32R, tag="xc")
                        for k2 in range(0, ET, 4):
                            k3 = min(k2 + 4, ET)
                            nc.sync.dma_start(
                                x_sb[:, k2:k3, :], xT_t[:, k2:k3, csl])
                    cos_c = acs.tile([P, CH], F32, tag="cos")
                    sin_c = acs.tile([P, CH], F32, tag="sin")
                    nc.gpsimd.dma_start(cos_c[:], cosF[:, csl])
                    nc.gpsimd.dma_start(sin_c[:], sinF[:, csl])
                    for w_sb, scr in ((wq_sb, qt_scr), (wk_sb, kt_scr)):
                        for pr in range(NPAIR):
                            mA, mB = 2 * pr, 2 * pr + 1
                            psA = aps.tile([P, CH], F32, tag="psA")
                            psB = aps.tile([P, CH], F32, tag="psB")
                            for k in range(ET):
                                nc.tensor.matmul(
                                    psA[:], w_sb[:, k, mA * P:(mA + 1) * P],
                                    x_sb[:, k, :], start=(k == 0), stop=(k == ET - 1))
                            for k in range(ET):
                                nc.tensor.matmul(
                                    psB[:], w_sb[:, k, mB * P:(mB + 1) * P],
                                    x_sb[:, k, :], start=(k == 0), stop=(k == ET - 1))
                            t1 = arot.tile([P, CH], F32, tag="t1")
                            t2 = arot.tile([P, CH], F32, tag="t2")
                            rA = arot.tile([P, CH], F32R, tag="rA")
                            nc.vector.tensor_tensor(t1[:], psA[:], cos_c[:], MUL)
                            nc.vector.tensor_tensor(t2[:], psB[:], sin_c[:], MUL)
                            nc.vector.tensor_tensor(rA[:], t1[:], t2[:], SUB)
                            t3 = arot.tile([P, CH], F32, tag="t1")
                            t4 = arot.tile([P, CH], F32, tag="t2")
                            rB = arot.tile([P, CH], F32R, tag="rA")
                            nc.vector.tensor_tensor(t3[:], psA[:], sin_c[:], MUL)
                            nc.vector.tensor_tensor(t4[:], psB[:], cos_c[:], MUL)
                            nc.vector.tensor_tensor(rB[:], t3[:], t4[:], ADD)
                            ha, hb = 2 * pr, 2 * pr + 1
                            nc.gpsimd.dma_start(scr[ha * HD:ha * HD + D2, csl], rA[0:D2, :])
                            nc.gpsimd.dma_start(scr[hb * HD:hb * HD + D2, csl], rA[D2:P, :])
                            nc.gpsimd.dma_start(scr[ha * HD + D2:(ha + 1) * HD, csl], rB[0:D2, :])
                            nc.gpsimd.dma_start(scr[hb * HD + D2:(hb + 1) * HD, csl], rB[D2:P, :])
                    for st in range(CH // P):
                        psV = aps.tile([P, HL * HD], F32, tag="psV")
                        for k in range(ET):
                            nc.tensor.matmul(
                                psV[:], x_sb[:, k, st * P:(st + 1) * P],
                                wv_sb[:, k, :], start=(k == 0), stop=(k == ET - 1))
                        v_sb = arot.tile([P, HL * HD], F32R, tag="vsb")
                        nc.scalar.copy(v_sb[:], psV[:])
                        s0 = c * CH + st * P
                        nc.gpsimd.dma_start(v_scr[s0:s0 + P, :], v_sb[:])
                    # prefetch head 0's K/V chunk for phase B right after this
                    # chunk's scatter lands (overlaps the rest of phase A)
                    nc.gpsimd.dma_start(kt0_sb[:, csl], kt_scr[0:HD, csl])
                    nc.gpsimd.dma_start(v0_sb[:, ssl, :], v_scr_t[:, ssl, 0:HD])
                    nc.gpsimd.dma_start(qt0_sb[:, csl], qt_scr[0:HD, csl])

            # ---------------- Phase B: attention per head ----------------
            with tc.tile_pool(name="battn", bufs=1) as battn:
                attn_sb = battn.tile([P, HL, T], F32R)
                wo_sb = battn.tile([P, HL, E], F32R)
                for h in range(HL):
                    nc.gpsimd.dma_start(wo_sb[:, h, :], wo_t[:, h, :])
                with (
                    tc.tile_pool(name="bh", bufs=2) as bh,
                    tc.tile_pool(name="bq", bufs=2) as bq,
                    tc.tile_pool(name="bpt", bufs=6) as bpt,
                    tc.tile_pool(name="bsp", bufs=4, space="PSUM") as bsp,
                    tc.tile_pool(name="bpv", bufs=2, space="PSUM") as bpv,
                    tc.tile_pool(name="brs", bufs=2, space="PSUM") as brs,
                ):
                    # K/V loaded one head ahead; head 0 uses the phase-A
                    # prefetch tiles.
                    kv_tiles = {0: (kt0_sb, v0_sb)}

                    def load_head(h):
                        hsl = slice(h * HD, (h + 1) * HD)
                        kt_sb = bh.tile([P, T], F32R, tag="kt")
                        v_sb = bh.tile([P, ST, HD], F32R, tag="vt")
                        nc.gpsimd.dma_start(kt_sb[:], kt_scr[hsl, :])
                        nc.gpsimd.dma_start(v_sb[:], v_scr_t[:, :, hsl])
                        kv_tiles[h] = (kt_sb, v_sb)

                    if HL > 1:
                        load_head(1)
                    for h in range(HL):
                        if h + 2 < HL:
                            load_head(h + 2)
                        hsl = slice(h * HD, (h + 1) * HD)
                        kt_sb, v_sb = kv_tiles.pop(h)
                        if h == 0:
                            qt_sb = qt0_sb
                        else:
                            qt_sb = bq.tile([P, T], F32R, tag="qt")
                            for c in range(TC):
                                csl = slice(c * CH, (c + 1) * CH)
                                nc.gpsimd.dma_start(qt_sb[:, csl], qt_scr[hsl, csl])
                        for c in range(TC):
                            csl = slice(c * CH, (c + 1) * CH)
                            pv = bpv.tile([P, CH], F32)
                            rs = brs.tile([P, CH], F32)
                            # DVE pre-sums adjacent pt pairs so only 8 rowsum
                            # matmuls are needed (DVE stays well under the PE
                            # time per chunk, unlike a full DVE reduction)
                            prev_pt = None
                            for st in range(ST):
                                sps = bsp.tile([P, CH], F32)
                                nc.tensor.matmul(
                                    sps[:], kt_sb[:, st * P:(st + 1) * P],
                                    qt_sb[:, csl], start=True, stop=True)
                                pt = bpt.tile([P, CH], F32R, tag="pt")
                                nc.scalar.activation(
                                    pt[:], sps[:], mybir.ActivationFunctionType.Exp,
                                    scale=scale)
                                nc.tensor.matmul(
                                    pv[:], v_sb[:, st, :], pt[:],
                                    start=(st == 0), stop=(st == ST - 1))
                                if st % 2 == 0:
                                    prev_pt = pt
                                else:
                                    psum2 = bpt.tile([P, CH], F32R, tag="ps2")
                                    nc.vector.tensor_tensor(
                                        psum2[:], prev_pt[:], pt[:], ADD)
                                    nc.tensor.matmul(
                                        rs[:], ones_sb[:], psum2[:],
                                        start=(st == 1), stop=(st == ST - 1))
                            rec = bpt.tile([P, CH], F32, tag="rec")
                            scr8 = bpt.tile([P, CH], F32, tag="scr8")
                            nc.vector.reciprocal_approx_accurate(
                                out=rec[:], in_=rs[:], scratch=scr8[:])
                            nc.vector.tensor_tensor(
                                attn_sb[:, h, csl], pv[:], rec[:], MUL)

                # ---------------- Phase C: output projection ----------------
                with (
                    tc.tile_pool(name="cout", bufs=4) as cout,
                    tc.tile_pool(name="cps", bufs=6, space="PSUM") as cps,
                ):
                    for tt in range(T // P):
                        tsl = slice(tt * P, (tt + 1) * P)
                        for oc in range(E // CH):
                            osl = slice(oc * CH, (oc + 1) * CH)
                            ops = cps.tile([P, CH], F32)
                            for h in range(HL):
                                nc.tensor.matmul(
                                    ops[:], attn_sb[:, h, tsl], wo_sb[:, h, osl],
                                    start=(h == 0), stop=(h == HL - 1))
                            o_sb = cout.tile([P, CH], F32)
                            if (tt + oc) % 2 == 0:
                                nc.vector.tensor_copy(o_sb[:], ops[:])
                                nc.sync.dma_start(outp[tsl, osl], o_sb[:])
                            else:
                                nc.scalar.copy(o_sb[:], ops[:])
                                nc.gpsimd.dma_start(outp[tsl, osl], o_sb[:])

    nc.finalize()
    return nc


# ---------------------------------------------------------------------------
# Host-side wrapper
# ---------------------------------------------------------------------------

_B, _T, _EMB = 2, 2048, 2048
_HQ, _HD = 16, 128
_GROUPS = 4                      # head groups; 2 batches x 4 groups = 8 cores
_HL = _HQ // _GROUPS             # 4 local heads per core

_nc_cache = {}


def _get_nc():
    key = (_T, _EMB, _HL, _HD)
    if key not in _nc_cache:
        _nc_cache[key] = build_attention_nc(_T, _EMB, _HL, _HD, CH=512)
    return _nc_cache[key]


def _prep_core_inputs(x, wq, wk, wv, wo, fc, fs):
    """Per-core input dicts for 8 cores (core = 4*batch + group)."""
    # RoPE pair-permutation within each head: [even dims, odd dims]
    perm = np.concatenate([np.arange(0, _HD, 2), np.arange(1, _HD, 2)])
    cosF = np.ascontiguousarray(np.tile(fc.T, (2, 1)), dtype=np.float32)  # [128,T]
    sinF = np.ascontiguousarray(np.tile(fs.T, (2, 1)), dtype=np.float32)

    xT = [np.ascontiguousarray(x[b].T) for b in range(_B)]

    in_maps = []
    for b in range(_B):
        for g in range(_GROUPS):
            heads = [g * _HL + h for h in range(_HL)]
            # device q/k row order: per pair (h0,h1): [h0_r, h1_r], [h0_i, h1_i]
            rows = []
            for pr in range(_HL // 2):
                h0, h1 = heads[2 * pr], heads[2 * pr + 1]
                for half in (perm[:64], perm[64:]):
                    rows.append(h0 * _HD + half)
                    rows.append(h1 * _HD + half)
            rows = np.concatenate(rows)
            vrows = np.concatenate([np.arange(h * _HD, (h + 1) * _HD) for h in heads])
            in_maps.append({
                "xT": xT[b],
                "wqT": np.ascontiguousarray(wq[rows].T),
                "wkT": np.ascontiguousarray(wk[rows].T),
                "wvT": np.ascontiguousarray(wv[vrows].T),
                "woT": np.ascontiguousarray(wo[:, vrows].T),
                "cosF": cosF,
                "sinF": sinF,
            })
    return in_maps


def kernel(**inputs):
    x = np.asarray(inputs["x"], dtype=np.float32)
    wq = np.asarray(inputs["wq"], dtype=np.float32)
    wk = np.asarray(inputs["wk"], dtype=np.float32)
    wv = np.asarray(inputs["wv"], dtype=np.float32)
    wo = np.asarray(inputs["wo"], dtype=np.float32)
    fc = np.asarray(inputs["freqs_cos"], dtype=np.float32)
    fs = np.asarray(inputs["freqs_sin"], dtype=np.float32)
    # start_pos is 0 (cache region [0, T) is fully overwritten before the read,
    # so k_cache/v_cache never contribute to the output).

    nc = _get_nc()
    in_maps = _prep_core_inputs(x, wq, wk, wv, wo, fc, fs)
    res = run_bass_kernel_spmd(nc, in_maps, core_ids=list(range(8)))

    out = np.empty((_B, _T, _EMB), dtype=np.float32)
    for b in range(_B):
        acc = np.zeros((_T, _EMB), dtype=np.float64)
        for g in range(_GROUPS):
            acc += res.results[4 * b + g]["outp"]
        out[b] = acc.astype(np.float32)
    return out

